# revision 7
# baseline (speedup 1.0000x reference)
"""Contrastive loss (supervised NT-Xent style) on 8 Trainium2 NeuronCores.

Reference (N=8192, D=256, C=64 classes, T=0.5):
    sim   = (E @ E.T) / T
    den'_i = sum_{j != i} exp(sim_ij)     (row-max subtraction cancels exactly)
    loss  = mean over positive pairs of (log den'_i - sim_ij)

v2: symmetric block scheme + fp8 DoubleRow matmuls + constant-bias exp.

Each core owns 1024 rows and computes exp(sim - 2) against only 5120 columns:
its own block plus partner blocks +1,+2,+3,+4 (mod 8).  Pairs at distance
1..3 are computed once; the partner's denominator contribution is recovered
as column sums (PE ones-matmul) of the exp block.  Distance-4 pairs are
computed in both orientations (row sums only).  Every row's denominator is
assembled on the host from one row-sum and three column-sum contributions.

exp uses a constant bias: exp(2*dot - 2) with dot in [-1, 1]; values live in
[e^-4, 1] so no per-row max is needed, which lets ACT tiles pack the PSUM
stream arbitrarily and makes column sums meaningful.

Per-core engines: PE fp8 DoubleRow sim matmuls (K=256 in one pass) + bf16
ones-matmul column sums; ACT exp (PSUM -> SBUF bf16, the bottleneck at
~38us); DVE row sums via tensor_scalar accum (4x mode on bf16) + column-sum
egress; host finalize.
"""

import numpy as np
import ml_dtypes

import concourse.bass as bass
import concourse.bacc as bacc
import concourse.mybir as mybir
import concourse.tile as tile
from concourse.bass_utils import run_bass_kernel_spmd

N = 8192
D = 256
C = 64
N_CORES = 8
M = N // N_CORES          # 1024 rows per core
P = 128                   # partitions
MT = M // P               # 8 m-tiles per core
CH = 512                  # matmul moving chunk / psum bank width
NG = 5                    # column groups per core (own, +1, +2, +3, +4)
SC = NG * M // CH         # 10 chunks of 512 cols per m-tile
COLS = NG * M             # 5120 moving columns per core
QW = 2048                 # ACT tile width (4 psum banks)
NPAIR = MT // 2           # 4 m-tile pairs
TPP = 5                   # QW tiles per pair (2*5120/2048)
CS0, CS1 = 2, 8           # chunk range [CS0, CS1) that gets column sums
NCS = CS1 - CS0           # 6 colsum chunks per m-tile
EIGHT = 8.0               # fp8 pre-scale: emb*8 keeps values in normal range

_F32 = mybir.dt.float32
_BF16 = mybir.dt.bfloat16
_F8 = mybir.dt.float8e4
_BF16_NP = ml_dtypes.bfloat16
_F8_NP = ml_dtypes.float8_e4m3


def build_nc(enable_asserts: bool = False):
    nc = bacc.Bacc(
        "TRN2",
        target_bir_lowering=False,
        debug=False,
        enable_asserts=enable_asserts,
        num_devices=N_CORES,
    )

    mov = nc.dram_tensor("mov", [SC, 2, P, CH], _F8, kind="ExternalInput").ap()
    lhsT = nc.dram_tensor("lhsT", [2, P, M], _F8, kind="ExternalInput").ap()
    emb_rows = nc.dram_tensor("emb_rows", [M, D], _BF16, kind="ExternalInput").ap()
    onehot_rows = nc.dram_tensor("onehot_rows", [M, C], _BF16, kind="ExternalInput").ap()

    # row_stats[:, 0:8] = rowsum per m-tile, [:, 8:16] = sumsq per m-tile
    row_stats_d = nc.dram_tensor("row_stats", [P, 2 * MT], _F32, kind="ExternalOutput").ap()
    # csum[0, pair*3072 + (s-2)*512 + x] = colsum of chunk s over the pair's rows
    csum_d = nc.dram_tensor("csum", [1, NPAIR * NCS * CH], _F32, kind="ExternalOutput").ap()
    g_part_d = nc.dram_tensor("g_part", [C, D], _F32, kind="ExternalOutput").ap()

    with tile.TileContext(nc) as tc:
        with (
            tc.tile_pool(name="big", bufs=1) as big,
            tc.tile_pool(name="small", bufs=1) as small,
            tc.tile_pool(name="psum", bufs=2, space=bass.MemorySpace.PSUM) as psum,
        ):
            # ---- persistent SBUF residents ----
            embT_sb = big.tile([P, 2, COLS], _F8, tag="embT")      # moving cols
            embTr_sb = big.tile([P, 2, M], _F8, tag="embTr")       # own rows (lhsT)
            embr_sb = big.tile([P, MT, D], _BF16, tag="embr")      # natural rows
            oh_sb = big.tile([P, MT, C], _BF16, tag="oh")          # onehot rows
            # exp ring: one m-tile pair of exp values, bf16
            exp_sb = [
                big.tile([P, 2 * COLS], _BF16, tag=f"exp{r}", name=f"exp_sb{r}")
                for r in range(2)
            ]

            row_stats = small.tile([P, 2 * MT], _F32, tag="rstats")
            cs_sb = small.tile([1, NPAIR * NCS * CH], _F32, tag="cssb")
            ones_sb = small.tile([P, 32], _BF16, tag="ones")
            sq_junk = small.tile([P, D], _F32, tag="sqjunk")
            g_sb = small.tile([C, D], _F32, tag="gsb")
            dummy = small.tile([P, 1], _F32, tag="dummy")
            warm = small.tile([P, P], _BF16, tag="warm")

            # ---- t=0: hoist the ACT exp table load; warm the PE HAM ----
            nc.gpsimd.memset(dummy[:], 0.0)
            nc.scalar.activation(
                out=dummy[:], in_=dummy[:],
                func=mybir.ActivationFunctionType.Exp, bias=0.0, scale=1.0,
            )
            nc.gpsimd.memset(warm[:], 0.0)
            nc.gpsimd.memset(ones_sb[:], 1.0)
            neg2 = small.tile([P, 1], _F32, tag="neg2")
            nc.gpsimd.memset(neg2[:], -2.0)
            warm_ps = psum.tile([P, P], _F32, tag="ps", name="warm_ps")
            for _ in range(24):
                nc.tensor.matmul(warm_ps[:], lhsT=warm[:], rhs=warm[:], start=True, stop=True)

            # ---- input DMAs (issue order == priority order) ----
            # sync queue: lhsT + first chunks; scalar queue (idle until the
            # first EXP): the rest; vector queue: rows + onehot.
            nc.sync.dma_start(out=embTr_sb[:, 0, :], in_=lhsT[0])
            nc.sync.dma_start(out=embTr_sb[:, 1, :], in_=lhsT[1])
            for s in range(5):
                for k in range(2):
                    nc.sync.dma_start(
                        out=embT_sb[:, k, s * CH:(s + 1) * CH], in_=mov[s, k]
                    )
            for s in range(5, SC):
                for k in range(2):
                    nc.scalar.dma_start(
                        out=embT_sb[:, k, s * CH:(s + 1) * CH], in_=mov[s, k]
                    )
            nc.gpsimd.dma_start(
                out=embr_sb[:],
                in_=emb_rows.rearrange("(m p) d -> p m d", p=P),
            )
            nc.gpsimd.dma_start(
                out=oh_sb[:],
                in_=onehot_rows.rearrange("(m p) c -> p m c", p=P),
            )

            # ---- per-row sumsq (bf16 rows; host uses it for the pos term) ----
            for m in range(MT):
                nc.vector.tensor_mul(sq_junk[:], embr_sb[:, m, :], embr_sb[:, m, :])
                nc.vector.tensor_reduce(
                    out=row_stats[:, MT + m:MT + m + 1],
                    in_=sq_junk[:],
                    axis=mybir.AxisListType.X,
                    op=mybir.AluOpType.add,
                )

            # ---- main loop over m-tile pairs ----
            for pair in range(NPAIR):
                ering = exp_sb[pair % 2]
                # sim matmuls (fp8 DoubleRow, K=256 in one pass) + exp
                for t in range(TPP):
                    ps = psum.tile([P, QW], _F32, tag="ps")
                    for j in range(QW // CH):
                        c = t * (QW // CH) + j
                        mloc, s = divmod(c, SC)
                        m = 2 * pair + mloc
                        nc.tensor.matmul(
                            ps[:, j * CH:(j + 1) * CH],
                            lhsT=embTr_sb[:, :, m * P:(m + 1) * P],
                            rhs=embT_sb[:, :, s * CH:(s + 1) * CH],
                            start=True,
                            stop=True,
                            perf_mode=mybir.MatmulPerfMode.DoubleRow,
                        )
                    # exp(2*dot - 2): psum holds 64*dot (inputs pre-scaled by 8)
                    nc.scalar.activation(
                        out=ering[:, t * QW:(t + 1) * QW],
                        in_=ps[:],
                        func=mybir.ActivationFunctionType.Exp,
                        bias=neg2[:],
                        scale=2.0 / (EIGHT * EIGHT),
                    )

                # row sums: DVE tensor_scalar 4x with accumulator, per m-tile
                for mloc in range(2):
                    m = 2 * pair + mloc
                    nc.vector.tensor_scalar(
                        out=ering[:, mloc * COLS:(mloc + 1) * COLS],
                        in0=ering[:, mloc * COLS:(mloc + 1) * COLS],
                        scalar1=1.0,
                        scalar2=None,
                        op0=mybir.AluOpType.mult,
                        op1=mybir.AluOpType.add,
                        accum_out=row_stats[:, m:m + 1],
                    )

                # column sums of chunks s=2..7 (partners +1,+2,+3) over the
                # pair's 256 rows: ones-matmul into 32-partition psum slots
                cs = psum.tile([P, QW], _F32, tag="ps")
                for idx in range(NCS):
                    s = CS0 + idx
                    out_sl = cs[32 * (idx // 4):32 * (idx // 4) + 32,
                                (idx % 4) * CH:(idx % 4 + 1) * CH]
                    for mloc in range(2):
                        nc.tensor.matmul(
                            out_sl,
                            lhsT=ones_sb[:, 0:32],
                            rhs=ering[:, mloc * COLS + s * CH:mloc * COLS + (s + 1) * CH],
                            start=(mloc == 0),
                            stop=(mloc == 1),
                        )
                # egress: copy the two result rows into the sbuf staging strip
                base = pair * NCS * CH
                nc.vector.tensor_copy(cs_sb[0:1, base:base + 4 * CH], cs[0:1, 0:4 * CH])
                nc.vector.tensor_copy(
                    cs_sb[0:1, base + 4 * CH:base + 6 * CH], cs[32:33, 0:2 * CH]
                )

            # ---- class sums over this core's rows: g[c, d] ----
            g_ps = psum.tile([C, D], _F32, tag="ps")
            for m in range(MT):
                nc.tensor.matmul(
                    g_ps[:],
                    lhsT=oh_sb[:, m, :],
                    rhs=embr_sb[:, m, :],
                    start=(m == 0),
                    stop=(m == MT - 1),
                )
            nc.vector.tensor_copy(g_sb[:], g_ps[:])
            nc.sync.dma_start(out=g_part_d[:], in_=g_sb[:])

            nc.sync.dma_start(out=csum_d[:], in_=cs_sb[:])
            nc.sync.dma_start(out=row_stats_d[:], in_=row_stats[:])

    nc.compile()
    return nc


_NC_CACHE = None


def _get_nc():
    global _NC_CACHE
    if _NC_CACHE is None:
        _NC_CACHE = build_nc()
    return _NC_CACHE


def make_in_maps(embeddings: np.ndarray, labels: np.ndarray):
    emb = np.asarray(embeddings, dtype=np.float32)
    labels = np.asarray(labels).astype(np.int64)
    emb16 = emb.astype(_BF16_NP)
    emb8 = (emb * EIGHT).astype(_F8_NP)          # pre-scaled fp8
    embT8 = np.ascontiguousarray(emb8.T)         # [D, N]
    onehot = (labels[:, None] == np.arange(C)[None, :]).astype(_BF16_NP)

    in_maps = []
    for c in range(N_CORES):
        r0, r1 = c * M, (c + 1) * M
        # moving columns: own block then partners +1..+4
        groups = [(c + g) % N_CORES for g in range(NG)]
        cols = np.concatenate(
            [embT8[:, g * M:(g + 1) * M] for g in groups], axis=1
        )  # [256, 5120]
        mv = np.ascontiguousarray(
            cols.reshape(2, P, SC, CH).transpose(2, 0, 1, 3)
        )  # [SC, 2, P, CH]
        lt = np.ascontiguousarray(embT8[:, r0:r1].reshape(2, P, M))
        in_maps.append(
            {
                "mov": mv,
                "lhsT": lt,
                "emb_rows": np.ascontiguousarray(emb16[r0:r1, :]),
                "onehot_rows": np.ascontiguousarray(onehot[r0:r1, :]),
            }
        )
    return in_maps


def finalize(results, embeddings: np.ndarray, labels: np.ndarray) -> np.float32:
    emb = np.asarray(embeddings, dtype=np.float32)
    labels = np.asarray(labels).astype(np.int64)
    emb8 = (emb * EIGHT).astype(_F8_NP).astype(np.float64) / EIGHT

    den2 = np.zeros(N, dtype=np.float64)   # sum_j exp(sim_ij - 2), j over all N
    sumsq = np.empty(N, dtype=np.float64)
    G = np.zeros((C, D), dtype=np.float64)
    for c in range(N_CORES):
        rs = np.asarray(results[c]["row_stats"], dtype=np.float64)  # [P, 16]
        for m in range(MT):
            rows = slice(c * M + m * P, c * M + (m + 1) * P)
            den2[rows] += rs[:, m]
            sumsq[rows] = rs[:, MT + m]
        cv = np.asarray(results[c]["csum"], dtype=np.float64).reshape(
            NPAIR, NCS, CH
        )
        for idx in range(NCS):
            pc = (c + 1 + idx // 2) % N_CORES
            j0 = pc * M + (idx % 2) * CH
            den2[j0:j0 + CH] += cv[:, idx, :].sum(axis=0)
        G += np.asarray(results[c]["g_part"], dtype=np.float64)

    # drop the diagonal term exp(2*||e8||^2 - 2) from each row's sum
    den2 -= np.exp(2.0 * (emb8 * emb8).sum(axis=1) - 2.0)
    logden = np.log(den2) + 2.0

    counts = np.bincount(labels, minlength=C)
    npos = counts[labels] - 1.0
    n_pos = npos.sum()
    pos_sim_total = 2.0 * ((G * G).sum() - sumsq.sum())
    numer = (npos * logden).sum() - pos_sim_total
    return np.float32(numer / n_pos)


def _run(inputs, trace: bool = False, **kwargs):
    nc = _get_nc()
    in_maps = make_in_maps(inputs["embeddings"], inputs["epitope_labels"])
    return run_bass_kernel_spmd(nc, in_maps, list(range(N_CORES)), trace=trace, **kwargs)


def kernel(embeddings, epitope_labels) -> np.ndarray:
    res = _run({"embeddings": embeddings, "epitope_labels": epitope_labels})
    return finalize(res.results, embeddings, epitope_labels)


# revision 15
# speedup vs baseline: 1.4051x; 1.4051x over previous
"""Contrastive loss (supervised NT-Xent style) on 8 Trainium2 NeuronCores.

Reference (N=8192, D=256, C=64 classes, T=0.5):
    sim   = (E @ E.T) / T
    den'_i = sum_{j != i} exp(sim_ij)     (row-max subtraction cancels exactly)
    loss  = mean over positive pairs of (log den'_i - sim_ij)

v2: symmetric block scheme + fp8 DoubleRow matmuls + constant-bias exp.

Each core owns 1024 rows and computes exp(sim - 2) against only 5120 columns:
its own block plus partner blocks +1,+2,+3,+4 (mod 8).  Pairs at distance
1..3 are computed once; the partner's denominator contribution is recovered
as column sums (PE ones-matmul) of the exp block.  Distance-4 pairs are
computed in both orientations (row sums only).  Every row's denominator is
assembled on the host from one row-sum and three column-sum contributions.

exp uses a constant bias: exp(2*dot - 2) with dot in [-1, 1]; values live in
[e^-4, 1] so no per-row max is needed, which lets ACT tiles pack the PSUM
stream arbitrarily and makes column sums meaningful.

Per-core engines: PE fp8 DoubleRow sim matmuls (K=256 in one pass) + bf16
ones-matmul column sums; ACT exp (PSUM -> SBUF bf16, the bottleneck at
~38us); DVE row sums via tensor_scalar accum (4x mode on bf16) + column-sum
egress; host finalize.
"""

import numpy as np
import ml_dtypes

import concourse.bass as bass
import concourse.bacc as bacc
import concourse.mybir as mybir
import concourse.tile as tile
from concourse.bass_utils import run_bass_kernel_spmd

N = 8192
D = 256
C = 64
N_CORES = 8
M = N // N_CORES          # 1024 rows per core
P = 128                   # partitions
MT = M // P               # 8 m-tiles per core
CH = 512                  # matmul moving chunk / psum bank width
NG = 5                    # column groups per core (own, +1, +2, +3, +4)
SC = NG * M // CH         # 10 chunks of 512 cols per m-tile
COLS = NG * M             # 5120 moving columns per core
QW = 2048                 # ACT tile width (4 psum banks)
NPAIR = MT // 2           # 4 m-tile pairs
TPP = 5                   # QW tiles per pair (2*5120/2048)
CS0, CS1 = 2, 8           # chunk range [CS0, CS1) that gets column sums
NCS = CS1 - CS0           # 6 colsum chunks per m-tile
EIGHT = 8.0               # fp8 pre-scale: emb*8 keeps values in normal range

_F32 = mybir.dt.float32
_BF16 = mybir.dt.bfloat16
_F8 = mybir.dt.float8e4
_BF16_NP = ml_dtypes.bfloat16
_F8_NP = ml_dtypes.float8_e4m3


def build_nc(enable_asserts: bool = False):
    nc = bacc.Bacc(
        "TRN2",
        target_bir_lowering=False,
        debug=False,
        enable_asserts=enable_asserts,
        num_devices=N_CORES,
    )

    # moving cols in 4 quarters of 1280 per k-tile: big per-partition lines
    mov = nc.dram_tensor("mov", [2, 4, P, COLS // 4], _F8, kind="ExternalInput").ap()
    lhsT = nc.dram_tensor("lhsT", [2, P, M], _F8, kind="ExternalInput").ap()
    emb_rows = nc.dram_tensor("emb_rows", [M, D], _BF16, kind="ExternalInput").ap()
    onehot_rows = nc.dram_tensor("onehot_rows", [M, C], _BF16, kind="ExternalInput").ap()

    # row_stats[:, 3m:3m+3] = rowsum partials per m-tile, [:, 24:32] = sumsq
    row_stats_d = nc.dram_tensor("row_stats", [P, 4 * MT], _F32, kind="ExternalOutput").ap()
    # csum[0, pair*3072 + (s-2)*512 + x] = colsum of chunk s over the pair's rows
    csum_d = nc.dram_tensor("csum", [1, NPAIR * NCS * CH], _F32, kind="ExternalOutput").ap()
    g_part_d = nc.dram_tensor("g_part", [C, D], _F32, kind="ExternalOutput").ap()

    with tile.TileContext(nc) as tc:
        with (
            tc.tile_pool(name="big", bufs=1) as big,
            tc.tile_pool(name="small", bufs=1) as small,
            tc.tile_pool(name="psum", bufs=2, space=bass.MemorySpace.PSUM) as psum,
        ):
            # ---- persistent SBUF residents ----
            embT_sb = big.tile([P, 2, COLS], _F8, tag="embT")      # moving cols
            embTr_sb = big.tile([P, 2, M], _F8, tag="embTr")       # own rows (lhsT)
            embr_sb = big.tile([P, MT, D], _BF16, tag="embr")      # natural rows
            oh_sb = big.tile([P, MT, C], _BF16, tag="oh")          # onehot rows
            # exp ring: one m-tile pair of exp values, bf16
            exp_sb = [
                big.tile([P, 2 * COLS], _BF16, tag=f"exp{r}", name=f"exp_sb{r}")
                for r in range(2)
            ]

            row_stats = small.tile([P, 4 * MT], _F32, tag="rstats")
            cs_sb = small.tile([1, NPAIR * NCS * CH], _F32, tag="cssb")
            ones_sb = small.tile([P, 32], _BF16, tag="ones")
            sq_junk = small.tile([P, D], _F32, tag="sqjunk")
            g_sb = small.tile([C, D], _F32, tag="gsb")
            dummy = small.tile([P, 1], _F32, tag="dummy")
            warm = small.tile([P, P], _BF16, tag="warm")

            # ---- t=0: hoist the ACT exp table load; warm the PE HAM ----
            nc.gpsimd.memset(dummy[:], 0.0)
            nc.scalar.activation(
                out=dummy[:], in_=dummy[:],
                func=mybir.ActivationFunctionType.Exp, bias=0.0, scale=1.0,
            )
            nc.gpsimd.memset(warm[:], 0.0)
            nc.gpsimd.memset(ones_sb[:], 1.0)
            neg2 = small.tile([P, 1], _F32, tag="neg2")
            nc.gpsimd.memset(neg2[:], -2.0)
            warm_ps = psum.tile([P, P], _F32, tag="ps", name="warm_ps")
            for _ in range(24):
                nc.tensor.matmul(warm_ps[:], lhsT=warm[:], rhs=warm[:], start=True, stop=True)

            # ---- input DMAs (issue order == priority order) ----
            # sync queue: lhsT + first chunks; scalar queue (idle until the
            # first EXP): the rest; vector queue: rows + onehot.
            nc.sync.dma_start(out=embTr_sb[:, 0, :], in_=lhsT[0])
            nc.sync.dma_start(out=embTr_sb[:, 1, :], in_=lhsT[1])
            QC = COLS // 4
            for q in range(4):
                for k in range(2):
                    eng = nc.sync if q < 2 else nc.scalar
                    eng.dma_start(
                        out=embT_sb[:, k, q * QC:(q + 1) * QC], in_=mov[k, q]
                    )
            nc.gpsimd.dma_start(
                out=embr_sb[:],
                in_=emb_rows.rearrange("(m p) d -> p m d", p=P),
            )
            nc.gpsimd.dma_start(
                out=oh_sb[:],
                in_=onehot_rows.rearrange("(m p) c -> p m c", p=P),
            )

            # ---- per-row sumsq (bf16 rows; host uses it for the pos term) ----
            for m in range(MT):
                nc.vector.tensor_mul(sq_junk[:], embr_sb[:, m, :], embr_sb[:, m, :])
                nc.vector.tensor_reduce(
                    out=row_stats[:, 3 * MT + m:3 * MT + m + 1],
                    in_=sq_junk[:],
                    axis=mybir.AxisListType.X,
                    op=mybir.AluOpType.add,
                )

            # ---- main loop over m-tile pairs ----
            # rowsum partial slots per m-tile: m_even -> [acc t0, acc t1,
            # dve t2-half]; m_odd -> [dve t2-half, acc t3, acc t4].
            ACC_SLOT = {0: 0, 1: 1, 3: 1, 4: 2}  # tile t -> slot (pure tiles)
            for pair in range(NPAIR):
                ering = exp_sb[pair % 2]
                # sim matmuls (fp8 DoubleRow, K=256 in one pass) + exp
                for t in range(TPP):
                    ps = psum.tile([P, QW], _F32, tag="ps")
                    for j in range(QW // CH):
                        c = t * (QW // CH) + j
                        mloc, s = divmod(c, SC)
                        m = 2 * pair + mloc
                        nc.tensor.matmul(
                            ps[:, j * CH:(j + 1) * CH],
                            lhsT=embTr_sb[:, :, m * P:(m + 1) * P],
                            rhs=embT_sb[:, :, s * CH:(s + 1) * CH],
                            start=True,
                            stop=True,
                            perf_mode=mybir.MatmulPerfMode.DoubleRow,
                        )
                    # exp(2*dot - 2): psum holds 64*dot (inputs pre-scaled by
                    # 8).  Pure tiles fuse their row-sum via accum_out.
                    if t != 2:
                        m = 2 * pair + (0 if t < 2 else 1)
                        acc = row_stats[:, 3 * m + ACC_SLOT[t]:3 * m + ACC_SLOT[t] + 1]
                    else:
                        acc = None
                    nc.scalar.activation(
                        out=ering[:, t * QW:(t + 1) * QW],
                        in_=ps[:],
                        func=mybir.ActivationFunctionType.Exp,
                        bias=neg2[:],
                        scale=2.0 / (EIGHT * EIGHT),
                        accum_out=acc,
                    )

                # mixed tile t2: per-m row-sum halves via DVE (read-only)
                for mloc in range(2):
                    m = 2 * pair + mloc
                    src = ering[:, mloc * COLS + (8 - 8 * mloc) * CH:
                                mloc * COLS + (8 - 8 * mloc) * CH + 2 * CH]
                    slot = 2 if mloc == 0 else 0
                    nc.vector.tensor_reduce(
                        out=row_stats[:, 3 * m + slot:3 * m + slot + 1],
                        in_=src,
                        axis=mybir.AxisListType.X,
                        op=mybir.AluOpType.add,
                    )

                # column sums of chunks s=2..7 (partners +1,+2,+3) over the
                # pair's 256 rows: ones-matmul into 32-partition psum slots
                cs = psum.tile([P, QW], _F32, tag="ps")
                for idx in range(NCS):
                    s = CS0 + idx
                    out_sl = cs[32 * (idx // 4):32 * (idx // 4) + 32,
                                (idx % 4) * CH:(idx % 4 + 1) * CH]
                    for mloc in range(2):
                        nc.tensor.matmul(
                            out_sl,
                            lhsT=ones_sb[:, 0:32],
                            rhs=ering[:, mloc * COLS + s * CH:mloc * COLS + (s + 1) * CH],
                            start=(mloc == 0),
                            stop=(mloc == 1),
                        )
                # egress: copy the two result rows into the sbuf staging strip
                base = pair * NCS * CH
                nc.vector.tensor_copy(cs_sb[0:1, base:base + 4 * CH], cs[0:1, 0:4 * CH])
                nc.vector.tensor_copy(
                    cs_sb[0:1, base + 4 * CH:base + 6 * CH], cs[32:33, 0:2 * CH]
                )

            # ---- class sums over this core's rows: g[c, d] ----
            g_ps = psum.tile([C, D], _F32, tag="ps")
            for m in range(MT):
                nc.tensor.matmul(
                    g_ps[:],
                    lhsT=oh_sb[:, m, :],
                    rhs=embr_sb[:, m, :],
                    start=(m == 0),
                    stop=(m == MT - 1),
                )
            nc.vector.tensor_copy(g_sb[:], g_ps[:])
            nc.sync.dma_start(out=g_part_d[:], in_=g_sb[:])

            nc.sync.dma_start(out=csum_d[:], in_=cs_sb[:])
            nc.sync.dma_start(out=row_stats_d[:], in_=row_stats[:])

    nc.compile()
    return nc


_NC_CACHE = None


def _get_nc():
    global _NC_CACHE
    if _NC_CACHE is None:
        _NC_CACHE = build_nc()
    return _NC_CACHE


def make_in_maps(embeddings: np.ndarray, labels: np.ndarray):
    emb = np.asarray(embeddings, dtype=np.float32)
    labels = np.asarray(labels).astype(np.int64)
    emb16 = emb.astype(_BF16_NP)
    emb8 = (emb * EIGHT).astype(_F8_NP)          # pre-scaled fp8
    embT8 = np.ascontiguousarray(emb8.T)         # [D, N]
    onehot = (labels[:, None] == np.arange(C)[None, :]).astype(_BF16_NP)

    in_maps = []
    for c in range(N_CORES):
        r0, r1 = c * M, (c + 1) * M
        # moving columns: own block then partners +1..+4
        groups = [(c + g) % N_CORES for g in range(NG)]
        cols = np.concatenate(
            [embT8[:, g * M:(g + 1) * M] for g in groups], axis=1
        )  # [256, 5120]
        mv = np.ascontiguousarray(
            cols.reshape(2, P, 4, COLS // 4).transpose(0, 2, 1, 3)
        )  # [2, 4, P, COLS//4]
        lt = np.ascontiguousarray(embT8[:, r0:r1].reshape(2, P, M))
        in_maps.append(
            {
                "mov": mv,
                "lhsT": lt,
                "emb_rows": np.ascontiguousarray(emb16[r0:r1, :]),
                "onehot_rows": np.ascontiguousarray(onehot[r0:r1, :]),
            }
        )
    return in_maps


def finalize(results, embeddings: np.ndarray, labels: np.ndarray) -> np.float32:
    emb = np.asarray(embeddings, dtype=np.float32)
    labels = np.asarray(labels).astype(np.int64)
    emb8 = (emb * EIGHT).astype(_F8_NP).astype(np.float64) / EIGHT

    den2 = np.zeros(N, dtype=np.float64)   # sum_j exp(sim_ij - 2), j over all N
    sumsq = np.empty(N, dtype=np.float64)
    G = np.zeros((C, D), dtype=np.float64)
    for c in range(N_CORES):
        rs = np.asarray(results[c]["row_stats"], dtype=np.float64)  # [P, 32]
        for m in range(MT):
            rows = slice(c * M + m * P, c * M + (m + 1) * P)
            den2[rows] += rs[:, 3 * m] + rs[:, 3 * m + 1] + rs[:, 3 * m + 2]
            sumsq[rows] = rs[:, 3 * MT + m]
        cv = np.asarray(results[c]["csum"], dtype=np.float64).reshape(
            NPAIR, NCS, CH
        )
        for idx in range(NCS):
            pc = (c + 1 + idx // 2) % N_CORES
            j0 = pc * M + (idx % 2) * CH
            den2[j0:j0 + CH] += cv[:, idx, :].sum(axis=0)
        G += np.asarray(results[c]["g_part"], dtype=np.float64)

    # drop the diagonal term exp(2*||e8||^2 - 2) from each row's sum
    den2 -= np.exp(2.0 * (emb8 * emb8).sum(axis=1) - 2.0)
    logden = np.log(den2) + 2.0

    counts = np.bincount(labels, minlength=C)
    npos = counts[labels] - 1.0
    n_pos = npos.sum()
    pos_sim_total = 2.0 * ((G * G).sum() - sumsq.sum())
    numer = (npos * logden).sum() - pos_sim_total
    return np.float32(numer / n_pos)


def _run(inputs, trace: bool = False, **kwargs):
    nc = _get_nc()
    in_maps = make_in_maps(inputs["embeddings"], inputs["epitope_labels"])
    return run_bass_kernel_spmd(nc, in_maps, list(range(N_CORES)), trace=trace, **kwargs)


def kernel(embeddings, epitope_labels) -> np.ndarray:
    res = _run({"embeddings": embeddings, "epitope_labels": epitope_labels})
    return finalize(res.results, embeddings, epitope_labels)
